# revision 31
# baseline (speedup 1.0000x reference)
"""Attention-pooling kernel for 8 Trainium2 NeuronCores.

Reference computation (per batch b):
    h      = tanh(emb @ W + bias)          # [T, 512]
    s      = tanh(h @ c)                   # [T]
    a      = softmax(s)                    # scores in [-1, 1] -> no max-sub needed
    pooled = sum_t a_t * emb[t]            # [512]
    out    = relu(pooled @ w1 + b1) @ w2 + b2

Strategy: pure data-parallel over batch (4 per core), emb pre-transposed on
host to [d, t] bf16 layout.  v2 engine-balance redesign:
  - scores c-reduction moved off the PE: DVE does y_mo = c_mo*tanh_mo
    (tensor_scalar @4x) + tree-adds (@2x); a single ones-stationary matmul
    (1T stream instead of 4T) does the 512->1 partition sum with its output
    replicated across all 128 partitions.
  - tanh/exp run on the replicated [128, FT] scores block, so exp's output
    IS the broadcast weight tile -> the gpsimd partition_broadcast is gone.
  - pooling uses fused tensor_tensor_reduce (mult+reduce in one DVE pass).
  - first/last iterations run as 512-wide sub-tiles to shorten the exposed
    startup / end-of-kernel chains; first-tile DMAs are split fine and spread
    across the SP and DVE rings.
"""

import os

import numpy as np
import ml_dtypes

B, T, D = 32, 4096, 512
H = 1024
DOUT = 8
NCORES = 8
BL = B // NCORES   # batches per core
P = 128
KD = D // P        # 4 contraction chunks for d=512
MO = D // P        # 4 output chunks for d_out=512
FT = 1024          # free-dim compute tile (t)
NFT = T // FT      # 4 compute tiles per batch
NS = NFT + 2       # accumulator slots per batch (first/last tiles run split)

_last_results = None  # stashed BassKernelResults for test.py profiling


def _build_graph():
    from contextlib import ExitStack

    import concourse.bass as bass
    import concourse.mybir as mybir
    import concourse.tile as tile
    from concourse.bacc import Bacc

    bf16 = mybir.dt.bfloat16
    f32 = mybir.dt.float32
    AF = mybir.ActivationFunctionType
    ALU = mybir.AluOpType

    nc = Bacc(None, target_bir_lowering=False)

    # embedding arrives pre-transposed from the host: emb[b, j, p, t] =
    # embedding[b, t, j*128+p] — so loads are plain contiguous DMAs
    emb = nc.declare_dram_parameter("emb", [BL, KD, P, T], bf16, isOutput=False)
    w_d = nc.declare_dram_parameter("w_sb", [P, KD, D], bf16, isOutput=False)
    c_d = nc.declare_dram_parameter("c_sb", [P, MO], f32, isOutput=False)
    b_d = nc.declare_dram_parameter("b_sb", [P, MO], f32, isOutput=False)
    w1_d = nc.declare_dram_parameter("w1_sb", [P, KD, H], bf16, isOutput=False)
    w2_d = nc.declare_dram_parameter("w2_sb", [P, H // P, DOUT], bf16, isOutput=False)
    b1r_d = nc.declare_dram_parameter("b1r_sb", [1, H], bf16, isOutput=False)
    out_d = nc.declare_dram_parameter("out", [DOUT, BL], f32, isOutput=True)
    den_d = nc.declare_dram_parameter("den", [1, BL], f32, isOutput=True)

    with tile.TileContext(nc) as tc, ExitStack() as ctx:
        const = ctx.enter_context(tc.tile_pool(name="const", bufs=1))
        embp = ctx.enter_context(tc.tile_pool(name="embp", bufs=4))
        hp = ctx.enter_context(tc.tile_pool(name="hp", bufs=2))
        yp = ctx.enter_context(tc.tile_pool(name="yp", bufs=2))
        yap = ctx.enter_context(tc.tile_pool(name="yap", bufs=2))
        ysp = ctx.enter_context(tc.tile_pool(name="ysp", bufs=2))
        wp = ctx.enter_context(tc.tile_pool(name="wp", bufs=2))
        scrp = ctx.enter_context(tc.tile_pool(name="scrp", bufs=2))
        clsp = ctx.enter_context(tc.tile_pool(name="clsp", bufs=1))
        zpsum = ctx.enter_context(tc.tile_pool(name="zpsum", bufs=2, space="PSUM"))
        spsum = ctx.enter_context(tc.tile_pool(name="spsum", bufs=1, space="PSUM"))
        cpsum = ctx.enter_context(tc.tile_pool(name="cpsum", bufs=1, space="PSUM"))

        # --- params into SBUF (once) ---
        w_t = const.tile([P, KD, D], bf16)
        c_t = const.tile([P, MO], f32)
        b_t = const.tile([P, MO], f32)
        w1_t = const.tile([P, KD, H], bf16)
        b1r_t = const.tile([1, H], bf16)
        w2_t = const.tile([P, H // P, DOUT], bf16)
        ones_t = const.tile([P, P], bf16)
        nc.gpsimd.memset(ones_t[:], 1.0)
        # w_t rides the ACT HWDGE ring first (the first matmul needs it);
        # the other params follow batch 0's bootstrap stripe on that ring
        nc.scalar.dma_start(out=w_t[:], in_=w_d[:])
        # dummy activation: forces the ~2.7us exp/tanh table load to happen
        # NOW, during the initial DMA wait, instead of ahead of the first
        # real tanh on the critical path
        warm = const.tile([1, 1], bf16)
        nc.scalar.activation(warm[:], ones_t[:1, :1], AF.Tanh)

        # accumulators: TTR seeds each slot (scalar=0.0), so only the
        # never-written slots need the memset; zeroing all is cheapest.
        pool_parts = const.tile([P, KD, BL, NS], f32)
        denoms = const.tile([P, BL, NS], f32)
        nc.vector.memset(pool_parts[:], 0.0)
        nc.vector.memset(denoms[:], 0.0)

        # --- software-pipelined main loop -------------------------------
        # Each iteration emits its h-matmul groups + tanh + c-mults + the
        # ysum tree immediately, but DEFERS the scores tail (ones-matmul,
        # tanh/exp, pooling STTs) until partway through the NEXT iteration's
        # h-matmul groups.  Without this the PE stalls ~3-5us per iteration
        # waiting for the DVE/gpsimd ysum chain.
        pending = []  # [(b, slot, t0, tw, ysum, embT, pact)]

        def flush_pending():
            while pending:
                (pb, pslot, pt0, ptw, pysum, pembT, pact) = pending.pop(0)
                sps = spsum.tile([P, FT], f32, tag="sps")
                for nh in range(max(ptw // 512, 1)):
                    ns = slice(nh * 512, min((nh + 1) * 512, ptw))
                    nc.tensor.matmul(
                        sps[:, ns], ones_t[:], pysum[:, ns], start=True, stop=True
                    )
                st = wp.tile([P, FT], bf16, tag="st")
                nc.scalar.activation(st[:, :ptw], sps[:, :ptw], AF.Tanh)
                wrep = wp.tile([P, FT], bf16, tag="wrep")
                nc.scalar.activation(
                    wrep[:, :ptw], st[:, :ptw], AF.Exp,
                    accum_out=denoms[:, pb, pslot:pslot + 1],
                )
                for j in range(KD):
                    # fused multiply + free-dim reduce in one DVE pass
                    # (tensor_tensor_reduce is broken on this HW/NRT — STT
                    # with accum_out computes the same thing)
                    scr = scrp.tile([P, FT], bf16, tag="scr")
                    if j >= KD - pact:
                        # DVE is the binding engine; hand the last chunk's
                        # reduction to ScalarE (plain multiply on DVE, then
                        # a Copy-activation accumulates along the free dim)
                        nc.vector.tensor_mul(
                            scr[:, :ptw], pembT[:, j, pt0:pt0 + ptw],
                            wrep[:, :ptw],
                        )
                        sink = scrp.tile([P, FT], bf16, tag="sink")
                        nc.scalar.activation(
                            sink[:, :ptw], scr[:, :ptw], AF.Copy,
                            accum_out=pool_parts[:, j, pb, pslot:pslot + 1],
                        )
                    else:
                        nc.vector.scalar_tensor_tensor(
                            out=scr[:, :ptw],
                            in0=pembT[:, j, pt0:pt0 + ptw],
                            scalar=1.0,
                            in1=wrep[:, :ptw],
                            op0=ALU.mult,
                            op1=ALU.mult,
                            accum_out=pool_parts[:, j, pb, pslot:pslot + 1],
                        )

        def attn_tile(b, slot, t0, tw, embT, pact=0):
            """h-matmul + scores ysum for one [t0, t0+tw) slice of batch b;
            the scores/pooling tail is queued on `pending`."""
            nhs = tw // 512
            full = tw == FT
            hT = hp.tile([P, MO, FT], bf16, tag="hT")
            y4 = yp.tile([P, MO, FT], bf16, tag="y4")
            ya = yap.tile([P, 2, FT], bf16, tag="ya")
            for mo in range(MO):
                zps = zpsum.tile([P, FT], f32, tag="zps")
                # ki outer / nh inner: each W chunk stays stationary for
                # both 512-halves (halves LDWEIGHTS traffic)
                for ki in range(KD):
                    for nh in range(max(nhs, 1)):
                        ns = slice(nh * 512, min((nh + 1) * 512, tw))
                        tsn = slice(t0 + ns.start, t0 + ns.stop)
                        nc.tensor.matmul(
                            zps[:, ns],
                            w_t[:, ki, mo * P:(mo + 1) * P],
                            embT[:, ki, tsn],
                            start=(ki == 0),
                            stop=(ki == KD - 1),
                        )
                nc.scalar.activation(
                    hT[:, mo, :tw], zps[:, :tw], AF.Tanh, bias=b_t[:, mo:mo + 1]
                )
                nc.vector.tensor_scalar_mul(
                    y4[:, mo, :tw], hT[:, mo, :tw], c_t[:, mo:mo + 1]
                )
                if mo == 1:
                    # y01 early, overlapping groups 2-3 (NOT on gpsimd: the
                    # gpsimd shares SBUF ports with the DVE and slows it)
                    nc.vector.tensor_add(
                        ya[:, 0, :tw], y4[:, 0, :tw], y4[:, 1, :tw]
                    )
            nc.vector.tensor_add(ya[:, 1, :tw], y4[:, 2, :tw], y4[:, 3, :tw])
            ysum = ysp.tile([P, FT], bf16, tag="ysum")
            nc.vector.tensor_add(ysum[:, :tw], ya[:, 0, :tw], ya[:, 1, :tw])
            # deferred tail of the PREVIOUS iteration: its ones-matmul lands
            # after this iteration's h-groups on the PE queue, by which time
            # its ysum chain has long resolved — the PE never stalls
            flush_pending()
            pending.append((b, slot, t0, tw, ysum, embT, pact))

        # 512-wide bootstrap tiles: the PE starts once the first 0.5 MiB
        # lands instead of waiting for a full 1 MiB tile
        plans = {
            0: [(0, 0, 512), (4, 512, 512),
                (1, FT, FT), (2, 2 * FT, FT), (3, 3 * FT, FT)],
        }
        for b in range(BL):
            embT = embp.tile([P, KD, T], bf16)  # embT[p, j, t] = emb[b, t, j*128+p]
            if b == 0:
                # bootstrap: the first full tile's 4 chunks land first,
                # split across the SP and ACT HWDGE rings (startup is DMA
                # latency-bound); params follow on the ACT ring
                for half in range(2):
                    for j in range(KD):
                        tsl = slice(half * 512, (half + 1) * 512)
                        eng = nc.sync if j < 2 else nc.scalar
                        eng.dma_start(out=embT[:, j, tsl], in_=emb[b, j, :, tsl])
                for dst, src in ((b_t, b_d), (c_t, c_d), (b1r_t, b1r_d)):
                    nc.scalar.dma_start(out=dst[:], in_=src[:])
                for tc_ in range(1, NFT):
                    tsl = slice(tc_ * FT, (tc_ + 1) * FT)
                    for j in range(KD):
                        nc.sync.dma_start(out=embT[:, j, tsl], in_=emb[b, j, :, tsl])
            else:
                for tc_ in range(NFT):
                    tsl = slice(tc_ * FT, (tc_ + 1) * FT)
                    for j in range(KD):
                        nc.sync.dma_start(out=embT[:, j, tsl], in_=emb[b, j, :, tsl])
            if b == 0:
                # classifier weights: needed only at the very end — load on
                # the SP ring behind batch 0 so the ACT sequencer stays free
                # for its first tanh
                nc.sync.dma_start(out=w1_t[:], in_=w1_d[:])
                nc.sync.dma_start(out=w2_t[:], in_=w2_d[:])
            for it, (slot, t0, tw) in enumerate(
                    plans.get(b, [(ti, ti * FT, FT) for ti in range(NFT)])):
                # hand pooling chunks to ScalarE (which has slack) where the
                # DVE is the binding engine: one chunk on middle iterations,
                # two on the final tile (its chain is the exposed tail)
                if b == BL - 1 and it == NFT - 1:
                    pact = 2
                elif 0 < b < BL - 1 or (b == BL - 1 and it == NFT - 2):
                    pact = 1
                else:
                    pact = 0
                attn_tile(b, slot, t0, tw, embT, pact=pact)
        flush_pending()

        # --- finalize: pooled = num / denom (division on host) ---
        pooledn = clsp.tile([P, KD, BL], f32)
        nc.vector.tensor_reduce(
            pooledn[:], pool_parts[:],
            axis=mybir.AxisListType.X, op=ALU.add,
        )
        dsum = clsp.tile([P, BL], f32)
        nc.vector.tensor_reduce(
            dsum[:], denoms[:],
            axis=mybir.AxisListType.X, op=ALU.add,
        )
        nc.sync.dma_start(out=den_d[:], in_=dsum[:1, :])
        dsum_bf = clsp.tile([1, BL], bf16)
        nc.vector.tensor_copy(dsum_bf[:1], dsum[:1, :])
        pooled_bf = clsp.tile([P, KD, BL], bf16)
        nc.vector.tensor_copy(pooled_bf[:], pooledn[:])

        # --- classifier on UNNORMALIZED pooled sums (host divides by den):
        # relu(num@w1/den + b1) == relu(num@w1 + den*b1)/den, so a K=1 matmul
        # adds den*b1 into the accumulation group ---
        r1 = clsp.tile([P, H // P, BL], bf16)
        for mo in range(H // P):
            c1ps = cpsum.tile([P, BL], f32, tag="c1ps")
            for ki in range(KD):
                nc.tensor.matmul(
                    c1ps[:],
                    w1_t[:, ki, mo * P:(mo + 1) * P],
                    pooled_bf[:, ki, :],
                    start=(ki == 0),
                    stop=False,
                )
            nc.tensor.matmul(
                c1ps[:],
                b1r_t[:1, mo * P:(mo + 1) * P],
                dsum_bf[:1],
                start=False,
                stop=True,
            )
            nc.scalar.activation(r1[:, mo], c1ps[:], AF.Relu)
        ops = cpsum.tile([DOUT, BL], f32, tag="ops")
        for ko in range(H // P):
            nc.tensor.matmul(
                ops[:],
                w2_t[:, ko, :],
                r1[:, ko, :],
                start=(ko == 0),
                stop=(ko == H // P - 1),
            )
        outsb = clsp.tile([DOUT, BL], f32)
        nc.vector.tensor_copy(outsb[:], ops[:])
        nc.sync.dma_start(out=out_d[:], in_=outsb[:])

    return nc


def kernel(**inputs) -> np.ndarray:
    global _last_results
    from concourse.bass_utils import run_bass_kernel_spmd

    emb = np.asarray(inputs["embedding"], dtype=np.float32)
    W = np.asarray(inputs["weight"], dtype=np.float32)
    bias = np.asarray(inputs["bias"], dtype=np.float32)
    c = np.asarray(inputs["context_weight"], dtype=np.float32)
    w1 = np.asarray(inputs["w1"], dtype=np.float32)
    b1 = np.asarray(inputs["b1"], dtype=np.float32)
    w2 = np.asarray(inputs["w2"], dtype=np.float32)
    b2 = np.asarray(inputs["b2"], dtype=np.float32)

    bf = ml_dtypes.bfloat16
    # pre-transpose on host: [B, T, D] -> [B, KD, P, T] so the device reads
    # contiguous partition rows instead of paying the DMA-xbar transpose
    emb_bf = np.ascontiguousarray(
        emb.astype(bf).reshape(B, T, KD, P).transpose(0, 2, 3, 1))
    w_sb = np.ascontiguousarray(
        W.reshape(KD, P, D).transpose(1, 0, 2)).astype(bf)
    c_sb = np.ascontiguousarray(c.reshape(MO, P).T).astype(np.float32)
    b_sb = np.ascontiguousarray(bias.reshape(MO, P).T).astype(np.float32)
    w1_sb = np.ascontiguousarray(
        w1.reshape(KD, P, H).transpose(1, 0, 2)).astype(bf)
    w2_sb = np.ascontiguousarray(
        w2.reshape(H // P, P, DOUT).transpose(1, 0, 2)).astype(bf)
    b1r_sb = b1.reshape(1, H).astype(bf)

    nc = _build_graph()
    if not nc.is_finalized():
        nc.finalize()
    in_maps = []
    for i in range(NCORES):
        in_maps.append({
            "emb": np.ascontiguousarray(emb_bf[i * BL:(i + 1) * BL]),
            "w_sb": w_sb, "c_sb": c_sb, "b_sb": b_sb,
            "w1_sb": w1_sb, "b1r_sb": b1r_sb, "w2_sb": w2_sb,
        })
    res = run_bass_kernel_spmd(
        nc, in_maps, core_ids=list(range(NCORES)),
        trace=bool(int(os.environ.get("KERNEL_TRACE", "0"))),
    )
    _last_results = res
    parts = []
    for i in range(NCORES):
        pre = np.asarray(res.results[i]["out"], np.float32).T   # [BL, DOUT]
        den = np.asarray(res.results[i]["den"], np.float32)[0]  # [BL]
        parts.append(pre / den[:, None] + b2[None, :])
    return np.concatenate(parts, axis=0).astype(np.float32)
